# revision 3
# baseline (speedup 1.0000x reference)
"""Trainium2 Bass kernel for nn_ArcEmbedding (embedding lookup + 3-axis RoPE).

Reference computation (per token t in batch b):
    e = emb_table[id]                       # [768]
    theta = [xn*invf, yn*invf, tn*invf]     # [384], xn = x/max(max_b(x),1) etc
    out[0:384]   = e[0:384]*cos(theta) - e[384:768]*sin(theta)
    out[384:768] = e[384:768]*cos(theta) + e[0:384]*sin(theta)

Kernel strategy (data-parallel over batch, 4 batches per NeuronCore, 8 cores):
  Polar refactor: with e1=e[0:384], e2=e[384:768],
      r_s  = sign(e1)*sqrt(e1^2+e2^2)
      phi0 = atan(e2/e1) in (-pi/2, pi/2)
      out[0:384]   = r_s * cos(phi0 + theta) = r_s * sin(pi/2 - phi0 - theta)
      out[384:768] = r_s * sin(phi0 + theta)
  Tiles are processed in PAIRS of 128 tokens with the pair interleaved
  host-side (tile A = even tokens, tile B = odd tokens of a 256-token
  block) so one output DMA writes 3072 contiguous bytes per partition.
  Per pair:
    - psi pair PSUM [128,1536] = [sinA|cosA|cosB|sinB] phase args, built
      by 6 bank-aligned matmuls against [phi||-phi] tables (B uses a
      column-reversed table so every matmul chunk stays inside a 2KB
      PSUM bank); the angle outer-product rows accumulate in PE PSUM
    - rg pair PSUM [128,1024] (two 384-col chunks, bank-aligned)
    - ONE ACT Sin over [128,1536] -> sin and cos halves simultaneously
      (args stay within ACT Sin's valid domain [-3.797, 3.797])
    - ONE DVE cast rg->bf16, then 3 DVE multiplies at 4x bf16 rate
    - bf16 output pair -> one 3KB/partition HWDGE DMA (host upcasts)
"""

import numpy as np

B, S, H, V = 32, 4096, 768, 64
P = 128
NCORES = 8
BPC = B // NCORES            # batches per core
NT = S // P                  # 128-token tiles per batch
NPAIR = NT // 2              # tile pairs per batch
HALF = H // 2                # 384
DA = HALF // 3               # 128 freqs per axis
ROPE_BASE = 10000.0

_INVF = (1.0 / (ROPE_BASE ** (np.arange(DA, dtype=np.float64) / DA))).astype(
    np.float32
)
_TNORM = (np.arange(S, dtype=np.float64) / (S - 1)).astype(np.float32)
# pair interleave: block of 256 tokens -> [even tokens | odd tokens]
_PERM = np.arange(S).reshape(NPAIR, P, 2).transpose(0, 2, 1).reshape(S)

_COMPILED = {}
LAST_RESULTS = None


def _build_program():
    import concourse.bacc as bacc
    import concourse.mybir as mybir
    import concourse.tile as tile

    f32 = mybir.dt.float32
    bf16 = mybir.dt.bfloat16
    AF = mybir.ActivationFunctionType
    ALU = mybir.AluOpType

    nc = bacc.Bacc("TRN2", target_bir_lowering=False, debug=False)

    ids_d = nc.dram_tensor("oh", [BPC, V, S], bf16, kind="ExternalInput")
    xyt_d = nc.dram_tensor("xyt", [BPC, 4, S], bf16, kind="ExternalInput")
    xymax_d = nc.dram_tensor("xymax", [BPC, P, 2 * NT], bf16, kind="ExternalInput")
    emb_d = nc.dram_tensor("emb", [V, H], f32, kind="ExternalInput")
    rtail_d = nc.dram_tensor("rhs_tail", [4, H], bf16, kind="ExternalInput")
    rtailr_d = nc.dram_tensor("rhs_tail_rev", [4, H], bf16, kind="ExternalInput")
    ident_d = nc.dram_tensor("ident", [P, P], f32, kind="ExternalInput")
    out_d = nc.dram_tensor("out", [BPC, S, H], bf16, kind="ExternalOutput")

    with tile.TileContext(nc) as tc:
        with (
            tc.tile_pool(name="const", bufs=1) as cpool,
            tc.tile_pool(name="batch", bufs=2) as bpool,
            tc.tile_pool(name="work", bufs=3) as wpool,
            tc.tile_pool(name="psum", bufs=2, space="PSUM") as ppool,
            tc.tile_pool(name="psum1", bufs=1, space="PSUM") as ppool1,
        ):
            # ---------------- one-time setup ----------------
            emb_sb = cpool.tile([V, H], f32)
            nc.sync.dma_start(out=emb_sb[:], in_=emb_d[:])
            ident_t = cpool.tile([P, P], f32)
            nc.sync.dma_start(out=ident_t[:], in_=ident_d[:])

            # rhs_psi  = [phi || -phi] (+ angle tail rows)
            # rhs_psiR = [-phi || phi] (reversed column halves, for the B
            # tile of each pair so all psum chunks stay bank-aligned)
            rhs_psi = cpool.tile([68, H], bf16)
            nc.vector.memset(rhs_psi[:], 0.0)
            nc.sync.dma_start(out=rhs_psi[64:68, :], in_=rtail_d[:])
            rhs_psir = cpool.tile([68, H], bf16)
            nc.vector.memset(rhs_psir[:], 0.0)
            nc.sync.dma_start(out=rhs_psir[64:68, :], in_=rtailr_d[:])
            rhs_r = cpool.tile([68, HALF], bf16)
            nc.vector.memset(rhs_r[:], 0.0)

            e1 = emb_sb[:, 0:HALF]
            e2 = emb_sb[:, HALF:H]
            sq1 = cpool.tile([V, HALF], f32)
            nc.vector.tensor_tensor(out=sq1[:], in0=e1, in1=e1, op=ALU.mult)
            sq2 = cpool.tile([V, HALF], f32)
            nc.vector.tensor_tensor(out=sq2[:], in0=e2, in1=e2, op=ALU.mult)
            ssum = cpool.tile([V, HALF], f32)
            nc.vector.tensor_tensor(out=ssum[:], in0=sq1[:], in1=sq2[:], op=ALU.add)
            rmag = cpool.tile([V, HALF], f32)
            nc.scalar.activation(out=rmag[:], in_=ssum[:], func=AF.Sqrt)
            neg = cpool.tile([V, HALF], f32)
            nc.vector.tensor_scalar(
                out=neg[:], in0=e1, scalar1=0.0, scalar2=None, op0=ALU.is_lt
            )
            sgn = cpool.tile([V, HALF], f32)
            nc.vector.tensor_scalar(
                out=sgn[:], in0=neg[:], scalar1=-2.0, scalar2=1.0,
                op0=ALU.mult, op1=ALU.add,
            )
            rsg = cpool.tile([V, HALF], f32)
            nc.vector.tensor_tensor(out=rsg[:], in0=rmag[:], in1=sgn[:], op=ALU.mult)
            nc.vector.tensor_copy(out=rhs_r[0:V, :], in_=rsg[:])

            einv = cpool.tile([V, HALF], f32)
            nc.vector.reciprocal(out=einv[:], in_=e1)
            quo = cpool.tile([V, HALF], f32)
            nc.vector.tensor_tensor(out=quo[:], in0=e2, in1=einv[:], op=ALU.mult)
            phi = cpool.tile([V, HALF], f32)
            nc.scalar.activation(out=phi[:], in_=quo[:], func=AF.Arctan)
            nc.vector.tensor_copy(out=rhs_psi[0:V, 0:HALF], in_=phi[:])
            nc.vector.tensor_scalar(
                out=rhs_psi[0:V, HALF:H], in0=phi[:], scalar1=-1.0, scalar2=None,
                op0=ALU.mult,
            )
            nc.vector.tensor_copy(out=rhs_psir[0:V, HALF:H], in_=phi[:])
            nc.vector.tensor_scalar(
                out=rhs_psir[0:V, 0:HALF], in0=phi[:], scalar1=-1.0, scalar2=None,
                op0=ALU.mult,
            )

            # ---------------- per batch ----------------
            for b in range(BPC):
                xyt_t = bpool.tile([4, S], bf16, tag="xyt")
                nc.sync.dma_start(out=xyt_t[:], in_=xyt_d[b])
                mxin = bpool.tile([P, 2 * NT], bf16, tag="mxin")
                nc.sync.dma_start(out=mxin[:], in_=xymax_d[b])
                mx2 = bpool.tile([P, 2], f32, tag="mx2")
                nc.vector.tensor_reduce(
                    out=mx2[:, 0:1], in_=mxin[:, 0:NT],
                    axis=mybir.AxisListType.X, op=ALU.max,
                )
                nc.vector.tensor_reduce(
                    out=mx2[:, 1:2], in_=mxin[:, NT:2 * NT],
                    axis=mybir.AxisListType.X, op=ALU.max,
                )
                # shares the rg slot (PSUM budget: 2*3 + 2 = 8 banks)
                pmx = ppool1.tile([2, P], f32, tag="rg")
                nc.tensor.transpose(out=pmx[:], in_=mx2[:], identity=ident_t[:])
                stg = bpool.tile([2, 4], f32, tag="stg")
                nc.vector.tensor_reduce(
                    out=stg[:, 0:1], in_=pmx[:],
                    axis=mybir.AxisListType.X, op=ALU.max,
                )
                nc.vector.tensor_scalar(
                    out=stg[:, 1:2], in0=stg[:, 0:1], scalar1=1.0,
                    scalar2=None, op0=ALU.max,
                )
                nc.vector.reciprocal(out=stg[:, 2:3], in_=stg[:, 1:2])
                # Pre-scale the x/y rows by 1/mx, 1/my (partitions 0-1),
                # then DMA them across partitions into the lhsT tiles.
                xys = bpool.tile([2, S], bf16, tag="xys")
                nc.vector.tensor_scalar(
                    out=xys[:], in0=xyt_t[0:2, :], scalar1=stg[:, 2:3],
                    scalar2=None, op0=ALU.mult,
                )
                L = bpool.tile([68, S], bf16, tag="bigL")
                nc.sync.dma_start(out=L[0:V, :], in_=ids_d[b])
                nc.sync.dma_start(out=L[64:66, :], in_=xys[:])
                nc.sync.dma_start(out=L[66:68, :], in_=xyt_d[b, 2:4])

                for jp in range(NPAIR):
                    w0 = jp * 2 * P           # pair start token (natural)
                    LA = L[:, w0:w0 + P]      # even tokens of the block
                    LB = L[:, w0 + P:w0 + 2 * P]  # odd tokens

                    # rg pair: [rA @0:384 | pad | rB @512:896 | pad]
                    # (both chunks bank-aligned inside a 2-bank tile)
                    rg = ppool1.tile([P, 1024], f32, tag="rg")
                    nc.tensor.matmul(
                        rg[:, 0:HALF], LA, rhs_r[:], start=True, stop=True,
                    )
                    nc.tensor.matmul(
                        rg[:, 512:512 + HALF], LB, rhs_r[:], start=True,
                        stop=True,
                    )
                    # psi pair: [sinA|cosA @0:768, cosB|sinB @768:1536]
                    psi = ppool.tile([P, 2 * H], f32, tag="psi")
                    nc.tensor.matmul(
                        psi[:, 0:512], LA, rhs_psi[:, 0:512],
                        start=True, stop=True,
                    )
                    nc.tensor.matmul(
                        psi[:, 512:768], LA, rhs_psi[:, 512:768],
                        start=True, stop=True,
                    )
                    nc.tensor.matmul(
                        psi[:, 768:1024], LB, rhs_psir[:, 0:256],
                        start=True, stop=True,
                    )
                    nc.tensor.matmul(
                        psi[:, 1024:1536], LB, rhs_psir[:, 256:768],
                        start=True, stop=True,
                    )

                    # one Sin for the whole pair:
                    # sc = [sinA(0:384)|cosA(384:768)|cosB(768:1152)|sinB(1152:1536)]
                    sc = wpool.tile([P, 2 * H], bf16, tag="sc")
                    nc.scalar.activation(out=sc[:], in_=psi[:], func=AF.Sin)

                    # one cast for the pair's amplitudes: rsb = [rA|rB] bf16
                    rsb = wpool.tile([P, H], bf16, tag="rsb")
                    rgv = rg[:].rearrange("p (b h) -> p b h", b=2, h=512)[
                        :, :, 0:HALF
                    ]
                    nc.vector.tensor_copy(
                        out=rsb[:].rearrange("p (b h) -> p b h", b=2), in_=rgv
                    )

                    # ot = [loA | hiA | loB | hiB], 384 cols each ->
                    # partition row = [token-even out | token-odd out]
                    ot = wpool.tile([P, 2 * H], bf16, tag="ot")
                    ot4 = ot[:].rearrange("p (b two h) -> p b two h", b=2, two=2)
                    # lo halves (r*cos): blocks @0,@768 <- cos blocks @384,@768
                    nc.vector.tensor_tensor(
                        out=ot4[:, :, 0, :],
                        in0=rsb[:].rearrange("p (b h) -> p b h", b=2),
                        in1=sc[:, HALF:HALF + 2 * HALF].rearrange(
                            "p (b h) -> p b h", b=2
                        ),
                        op=ALU.mult,
                    )
                    # hi halves (r*sin), one per tile (sin blocks @0, @1152)
                    nc.vector.tensor_tensor(
                        out=ot[:, HALF:2 * HALF], in0=rsb[:, 0:HALF],
                        in1=sc[:, 0:HALF], op=ALU.mult,
                    )
                    nc.vector.tensor_tensor(
                        out=ot[:, 3 * HALF:4 * HALF], in0=rsb[:, HALF:H],
                        in1=sc[:, 3 * HALF:4 * HALF], op=ALU.mult,
                    )
                    nc.sync.dma_start(
                        out=out_d[b, w0:w0 + 2 * P, :].rearrange(
                            "(p k) h -> p (k h)", k=2
                        ),
                        in_=ot[:],
                    )

    nc.compile()
    return nc


def _host_inputs(input_ids, coords, emb_table):
    import ml_dtypes

    bf16 = ml_dtypes.bfloat16
    ids = np.asarray(input_ids).astype(np.float32)[:, _PERM]     # [B, S]
    xy = np.asarray(coords).astype(np.float32)[:, _PERM, :]      # [B, S, 2]
    emb = np.asarray(emb_table).astype(np.float32)               # [V, H]
    tnorm = _TNORM[_PERM]

    ident = np.eye(P, dtype=np.float32)
    rtail = np.zeros((4, H), dtype=np.float32)
    rtail[0, 0:DA] = _INVF                                   # x row, sin half
    rtail[0, HALF:HALF + DA] = -_INVF                        # x row, cos half
    rtail[1, DA:2 * DA] = _INVF                              # y row, sin half
    rtail[1, HALF + DA:HALF + 2 * DA] = -_INVF               # y row, cos half
    rtail[2, 2 * DA:HALF] = _INVF                            # t row, sin half
    rtail[2, HALF + 2 * DA:H] = -_INVF                       # t row, cos half
    rtail[3, HALF:H] = np.pi / 2                             # ones row, cos half
    rtailr = np.concatenate([rtail[:, HALF:H], rtail[:, 0:HALF]], axis=1)
    rtail = rtail.astype(bf16)
    rtailr = rtailr.astype(bf16)

    in_maps = []
    for c in range(NCORES):
        bs = slice(c * BPC, (c + 1) * BPC)
        oh = (
            ids[bs][:, None, :] == np.arange(V, dtype=np.float32)[None, :, None]
        ).astype(bf16)                                       # [BPC, V, S]
        xyt = np.empty((BPC, 4, S), dtype=np.float32)
        xyt[:, 0, :] = xy[bs, :, 0]
        xyt[:, 1, :] = xy[bs, :, 1]
        xyt[:, 2, :] = tnorm[None, :]
        xyt[:, 3, :] = 1.0
        xymax = np.empty((BPC, P, 2 * NT), dtype=np.float32)
        xymax[:, :, 0:NT] = xy[bs, :, 0].reshape(BPC, NT, P).transpose(0, 2, 1)
        xymax[:, :, NT:2 * NT] = (
            xy[bs, :, 1].reshape(BPC, NT, P).transpose(0, 2, 1)
        )
        in_maps.append(
            {
                "oh": oh,
                "xyt": xyt.astype(bf16),
                "xymax": xymax.astype(bf16),
                "emb": emb,
                "rhs_tail": rtail,
                "rhs_tail_rev": rtailr,
                "ident": ident,
            }
        )
    return in_maps


def kernel(input_ids, coords, emb_table):
    global LAST_RESULTS
    from concourse.bass_utils import run_bass_kernel_spmd

    if "nc" not in _COMPILED:
        _COMPILED["nc"] = _build_program()
    nc = _COMPILED["nc"]

    in_maps = _host_inputs(input_ids, coords, emb_table)
    res = run_bass_kernel_spmd(nc, in_maps, core_ids=list(range(NCORES)))
    LAST_RESULTS = res
    out = np.concatenate(
        [r["out"].astype(np.float32) for r in res.results], axis=0
    )
    return out


# revision 8
# speedup vs baseline: 1.1121x; 1.1121x over previous
"""Trainium2 Bass kernel for nn_ArcEmbedding (embedding lookup + 3-axis RoPE).

Reference computation (per token t in batch b):
    e = emb_table[id]                       # [768]
    theta = [xn*invf, yn*invf, tn*invf]     # [384], xn = x/max(max_b(x),1) etc
    out[0:384]   = e[0:384]*cos(theta) - e[384:768]*sin(theta)
    out[384:768] = e[384:768]*cos(theta) + e[0:384]*sin(theta)

Kernel strategy (data-parallel over batch, 4 batches per NeuronCore, 8 cores):
  Polar refactor: with e1=e[0:384], e2=e[384:768],
      r_s  = sign(e1)*sqrt(e1^2+e2^2)
      phi0 = atan(e2/e1) in (-pi/2, pi/2)
      psi  = phi0 + theta                  (one 384-col matmul per tile)
      out[0:384]   = r_s * cos(psi) = r_s * sin(-psi + pi/2)
      out[384:768] = r_s * sin(psi)
  The cos comes from the SAME psum as the sin using the ACT engine's
  scale/bias (sin(-x + pi/2)), so the PE only streams 384 psi columns
  plus 384 amplitude columns per 128-token tile.
  Tiles are processed in PAIRS with the pair interleaved host-side
  (tile A = even tokens, tile B = odd tokens of a 256-token block) so
  one output DMA writes 3072 contiguous bytes per partition.
  Per pair: 4 matmuls (psi A/B + rg A/B, all bank-aligned), 2 ACT Sin
  ops (sin and cos for both tiles at once), 1 gpsimd cast of the
  amplitudes, 2 DVE bf16 multiplies, 1 output DMA.
  All per-batch preambles (normalization maxes, scaled coord rows, the
  one-hot DMAs) are hoisted before the pair loops so the engines never
  stall at batch boundaries.
"""

import numpy as np

B, S, H, V = 32, 4096, 768, 64
P = 128
NCORES = 8
BPC = B // NCORES            # batches per core
NT = S // P                  # 128-token tiles per batch
NPAIR = NT // 2              # tile pairs per batch
HALF = H // 2                # 384
DA = HALF // 3               # 128 freqs per axis
ROPE_BASE = 10000.0

_INVF = (1.0 / (ROPE_BASE ** (np.arange(DA, dtype=np.float64) / DA))).astype(
    np.float32
)
_TNORM = (np.arange(S, dtype=np.float64) / (S - 1)).astype(np.float32)
# pair interleave: block of 256 tokens -> [even tokens | odd tokens]
_PERM = np.arange(S).reshape(NPAIR, P, 2).transpose(0, 2, 1).reshape(S)

_COMPILED = {}
LAST_RESULTS = None


def _build_program():
    import concourse.bacc as bacc
    import concourse.mybir as mybir
    import concourse.tile as tile

    f32 = mybir.dt.float32
    bf16 = mybir.dt.bfloat16
    AF = mybir.ActivationFunctionType
    ALU = mybir.AluOpType

    nc = bacc.Bacc("TRN2", target_bir_lowering=False, debug=False)

    ids_d = nc.dram_tensor("oh", [BPC, V, S], bf16, kind="ExternalInput")
    xyt_d = nc.dram_tensor("xyt", [BPC, 3, S], bf16, kind="ExternalInput")
    xymax_d = nc.dram_tensor("xymax", [BPC, P, 2 * NT], bf16, kind="ExternalInput")
    emb_d = nc.dram_tensor("emb", [V, H], f32, kind="ExternalInput")
    rtail_d = nc.dram_tensor("rhs_tail", [3, HALF], bf16, kind="ExternalInput")
    ident_d = nc.dram_tensor("ident", [P, P], f32, kind="ExternalInput")
    out_d = nc.dram_tensor("out", [BPC, S, H], bf16, kind="ExternalOutput")

    with tile.TileContext(nc) as tc:
        with (
            tc.tile_pool(name="const", bufs=1) as cpool,
            tc.tile_pool(name="batch", bufs=BPC) as bpool,
            tc.tile_pool(name="work", bufs=4) as wpool,
            tc.tile_pool(name="psum", bufs=2, space="PSUM") as ppool,
        ):
            # ---------------- one-time setup ----------------
            emb_sb = cpool.tile([V, H], f32)
            nc.sync.dma_start(out=emb_sb[:], in_=emb_d[:])
            ident_t = cpool.tile([P, P], f32)
            nc.sync.dma_start(out=ident_t[:], in_=ident_d[:])

            # one combined stationary-side table: [psi cols 0:384 | r cols 384:768]
            # rows 0:64 = phi / r_s gather rows, rows 64:67 = x/y/t angle rows
            rhs_t = cpool.tile([67, H], bf16)
            nc.vector.memset(rhs_t[:], 0.0)
            nc.sync.dma_start(out=rhs_t[64:67, 0:HALF], in_=rtail_d[:])
            halfpi = cpool.tile([P, 1], f32)
            nc.vector.memset(halfpi[:], float(np.pi / 2))

            e1 = emb_sb[:, 0:HALF]
            e2 = emb_sb[:, HALF:H]
            sq1 = cpool.tile([V, HALF], f32)
            nc.vector.tensor_tensor(out=sq1[:], in0=e1, in1=e1, op=ALU.mult)
            sq2 = cpool.tile([V, HALF], f32)
            nc.vector.tensor_tensor(out=sq2[:], in0=e2, in1=e2, op=ALU.mult)
            ssum = cpool.tile([V, HALF], f32)
            nc.vector.tensor_tensor(out=ssum[:], in0=sq1[:], in1=sq2[:], op=ALU.add)
            rmag = cpool.tile([V, HALF], f32)
            nc.scalar.activation(out=rmag[:], in_=ssum[:], func=AF.Sqrt)
            neg = cpool.tile([V, HALF], f32)
            nc.vector.tensor_scalar(
                out=neg[:], in0=e1, scalar1=0.0, scalar2=None, op0=ALU.is_lt
            )
            sgn = cpool.tile([V, HALF], f32)
            nc.vector.tensor_scalar(
                out=sgn[:], in0=neg[:], scalar1=-2.0, scalar2=1.0,
                op0=ALU.mult, op1=ALU.add,
            )
            rsg = cpool.tile([V, HALF], f32)
            nc.vector.tensor_tensor(out=rsg[:], in0=rmag[:], in1=sgn[:], op=ALU.mult)
            nc.vector.tensor_copy(out=rhs_t[0:V, HALF:H], in_=rsg[:])

            einv = cpool.tile([V, HALF], f32)
            nc.vector.reciprocal(out=einv[:], in_=e1)
            quo = cpool.tile([V, HALF], f32)
            nc.vector.tensor_tensor(out=quo[:], in0=e2, in1=einv[:], op=ALU.mult)
            phi = cpool.tile([V, HALF], f32)
            nc.scalar.activation(out=phi[:], in_=quo[:], func=AF.Arctan)
            nc.vector.tensor_copy(out=rhs_t[0:V, 0:HALF], in_=phi[:])

            # ---------------- all per-batch preambles, hoisted ----------------
            Ls = []
            for b in range(BPC):
                xyt_t = bpool.tile([3, S], bf16, tag="xyt")
                nc.sync.dma_start(out=xyt_t[:], in_=xyt_d[b])
                mxin = bpool.tile([P, 2 * NT], bf16, tag="mxin")
                nc.sync.dma_start(out=mxin[:], in_=xymax_d[b])
                mx2 = bpool.tile([P, 2], f32, tag="mx2")
                nc.vector.tensor_reduce(
                    out=mx2[:, 0:1], in_=mxin[:, 0:NT],
                    axis=mybir.AxisListType.X, op=ALU.max,
                )
                nc.vector.tensor_reduce(
                    out=mx2[:, 1:2], in_=mxin[:, NT:2 * NT],
                    axis=mybir.AxisListType.X, op=ALU.max,
                )
                # shares the rg psum slots (PSUM budget: 2*2 + 2*2 = 8 banks)
                pmx = ppool.tile([2, P], f32, tag="rg")
                nc.tensor.transpose(out=pmx[:], in_=mx2[:], identity=ident_t[:])
                stg = bpool.tile([2, 4], f32, tag="stg")
                nc.vector.tensor_reduce(
                    out=stg[:, 0:1], in_=pmx[:],
                    axis=mybir.AxisListType.X, op=ALU.max,
                )
                nc.vector.tensor_scalar(
                    out=stg[:, 1:2], in0=stg[:, 0:1], scalar1=1.0,
                    scalar2=None, op0=ALU.max,
                )
                nc.vector.reciprocal(out=stg[:, 2:3], in_=stg[:, 1:2])
                # Pre-scale the x/y rows by 1/mx, 1/my (partitions 0-1),
                # then DMA them across partitions into the lhsT tiles.
                xys = bpool.tile([2, S], bf16, tag="xys")
                nc.vector.tensor_scalar(
                    out=xys[:], in0=xyt_t[0:2, :], scalar1=stg[:, 2:3],
                    scalar2=None, op0=ALU.mult,
                )
                L = bpool.tile([67, S], bf16, tag="bigL")
                nc.sync.dma_start(out=L[0:V, :], in_=ids_d[b])
                nc.sync.dma_start(out=L[64:66, :], in_=xys[:])
                nc.sync.dma_start(out=L[66:67, :], in_=xyt_d[b, 2:3])
                Ls.append(L)

            # ---------------- main loop: pure pair work ----------------
            for b in range(BPC):
                L = Ls[b]
                for jp in range(NPAIR):
                    w0 = jp * 2 * P           # pair start token (natural)
                    LA = L[:, w0:w0 + P]      # even tokens of the block
                    LB = L[:, w0 + P:w0 + 2 * P]  # odd tokens

                    # psi pair: [psiA @0:384 | pad | psiB @512:896 | pad]
                    psi = ppool.tile([P, 1024], f32, tag="psi")
                    nc.tensor.matmul(
                        psi[:, 0:HALF], LA, rhs_t[:, 0:HALF],
                        start=True, stop=True,
                    )
                    nc.tensor.matmul(
                        psi[:, 512:512 + HALF], LB, rhs_t[:, 0:HALF],
                        start=True, stop=True,
                    )
                    # rg pair: same layout, r_s gather columns
                    rg = ppool.tile([P, 1024], f32, tag="rg")
                    nc.tensor.matmul(
                        rg[:, 0:HALF], LA, rhs_t[:, HALF:H],
                        start=True, stop=True,
                    )
                    nc.tensor.matmul(
                        rg[:, 512:512 + HALF], LB, rhs_t[:, HALF:H],
                        start=True, stop=True,
                    )

                    psiv = psi[:].rearrange("p (b h) -> p b h", b=2, h=512)[
                        :, :, 0:HALF
                    ]
                    # sin and cos for the whole pair from the same psum:
                    # snP = [sinA|sinB], csP = [cosA|cosB] (cos = sin(-x+pi/2))
                    snp = wpool.tile([P, H], bf16, tag="snp")
                    nc.scalar.activation(
                        out=snp[:].rearrange("p (b h) -> p b h", b=2),
                        in_=psiv, func=AF.Sin,
                    )
                    csp = wpool.tile([P, H], bf16, tag="csp")
                    nc.scalar.activation(
                        out=csp[:].rearrange("p (b h) -> p b h", b=2),
                        in_=psiv, func=AF.Sin, scale=-1.0, bias=halfpi[:],
                    )

                    # amplitude cast; gpsimd can't read PSUM, so balance the
                    # f32 cast between DVE (most pairs) and ACT (every 5th)
                    rsb = wpool.tile([P, H], bf16, tag="rsb")
                    rgv = rg[:].rearrange("p (b h) -> p b h", b=2, h=512)[
                        :, :, 0:HALF
                    ]
                    rsbo = rsb[:].rearrange("p (b h) -> p b h", b=2)
                    if jp % 5 == 4:
                        nc.scalar.copy(out=rsbo, in_=rgv)
                    else:
                        nc.vector.tensor_copy(out=rsbo, in_=rgv)

                    # ot = [loA | hiA | loB | hiB], 384 cols each ->
                    # partition row = [token-even out | token-odd out]
                    ot = wpool.tile([P, 2 * H], bf16, tag="ot")
                    ot4 = ot[:].rearrange("p (b two h) -> p b two h", b=2, two=2)
                    rsbv = rsb[:].rearrange("p (b h) -> p b h", b=2)
                    nc.vector.tensor_tensor(
                        out=ot4[:, :, 0, :], in0=rsbv,
                        in1=csp[:].rearrange("p (b h) -> p b h", b=2),
                        op=ALU.mult,
                    )
                    nc.vector.tensor_tensor(
                        out=ot4[:, :, 1, :], in0=rsbv,
                        in1=snp[:].rearrange("p (b h) -> p b h", b=2),
                        op=ALU.mult,
                    )
                    nc.sync.dma_start(
                        out=out_d[b, w0:w0 + 2 * P, :].rearrange(
                            "(p k) h -> p (k h)", k=2
                        ),
                        in_=ot[:],
                    )

    nc.compile()
    return nc


def _host_inputs(input_ids, coords, emb_table):
    import ml_dtypes

    bf16 = ml_dtypes.bfloat16
    ids = np.asarray(input_ids).astype(np.float32)[:, _PERM]     # [B, S]
    xy = np.asarray(coords).astype(np.float32)[:, _PERM, :]      # [B, S, 2]
    emb = np.asarray(emb_table).astype(np.float32)               # [V, H]
    tnorm = _TNORM[_PERM]

    ident = np.eye(P, dtype=np.float32)
    rtail = np.zeros((3, HALF), dtype=np.float32)
    rtail[0, 0:DA] = _INVF                                   # x angle row
    rtail[1, DA:2 * DA] = _INVF                              # y angle row
    rtail[2, 2 * DA:HALF] = _INVF                            # t angle row
    rtail = rtail.astype(bf16)

    in_maps = []
    for c in range(NCORES):
        bs = slice(c * BPC, (c + 1) * BPC)
        oh = (
            ids[bs][:, None, :] == np.arange(V, dtype=np.float32)[None, :, None]
        ).astype(bf16)                                       # [BPC, V, S]
        xyt = np.empty((BPC, 3, S), dtype=np.float32)
        xyt[:, 0, :] = xy[bs, :, 0]
        xyt[:, 1, :] = xy[bs, :, 1]
        xyt[:, 2, :] = tnorm[None, :]
        xymax = np.empty((BPC, P, 2 * NT), dtype=np.float32)
        xymax[:, :, 0:NT] = xy[bs, :, 0].reshape(BPC, NT, P).transpose(0, 2, 1)
        xymax[:, :, NT:2 * NT] = (
            xy[bs, :, 1].reshape(BPC, NT, P).transpose(0, 2, 1)
        )
        in_maps.append(
            {
                "oh": oh,
                "xyt": xyt.astype(bf16),
                "xymax": xymax.astype(bf16),
                "emb": emb,
                "rhs_tail": rtail,
                "ident": ident,
            }
        )
    return in_maps


def kernel(input_ids, coords, emb_table):
    global LAST_RESULTS
    from concourse.bass_utils import run_bass_kernel_spmd

    if "nc" not in _COMPILED:
        _COMPILED["nc"] = _build_program()
    nc = _COMPILED["nc"]

    in_maps = _host_inputs(input_ids, coords, emb_table)
    res = run_bass_kernel_spmd(nc, in_maps, core_ids=list(range(NCORES)))
    LAST_RESULTS = res
    out = np.concatenate(
        [r["out"].astype(np.float32) for r in res.results], axis=0
    )
    return out
